# revision 1
# baseline (speedup 1.0000x reference)
"""Trainium2 Bass kernel for nn_DTransformer_10909216932644.

Sharding: 8 cores = 4 batches x 2 sequence halves. Feature-major (transposed)
activations [D, T_local]. Attention is head-split within each pair (8 heads
per core over the full 2048-token sequence, causal blocks only), using pair
AllGather + per-core permuted QKV weights + 0/1 blend scalars so the single
SPMD program is rank-agnostic.

Dtypes: float32r (TF32-like, full PE rate at N>=256) for GEMMs/residual/LN;
bf16 for the attention K/Q/V/probs path, comm buffers, and the MLP down proj.
"""

import sys

sys.path.insert(0, "/opt/trn_rl_repo")

import numpy as np
import ml_dtypes

import concourse.bass as bass
import concourse.tile as tile
from concourse import bacc, mybir
from concourse.bass_utils import run_bass_kernel_spmd

F32 = mybir.dt.float32
F32R = mybir.dt.float32r
BF16 = mybir.dt.bfloat16
AF = mybir.ActivationFunctionType
OP = mybir.AluOpType

L = 8
D = 1024
H = 16
HD = 64
M = 4096
V = 64
B, T = 4, 2048
TL = 1024
EPS = 1e-5
NLH = 8
NP = 4
NC = 8
DC = 8
NQT = 4

_CACHE = {}


def build_program(n_layers=L, repeat=1, skip=()):
    nc = bacc.Bacc("TRN2", target_bir_lowering=False, debug=False, num_devices=NC)

    toksT = nc.dram_tensor("toksT", [V, TL], F32R, kind="ExternalInput")
    posT = nc.dram_tensor("posT", [D, TL], F32R, kind="ExternalInput")
    wtokT = nc.dram_tensor("wtokT", [V, D], F32R, kind="ExternalInput")
    wqkvT = nc.dram_tensor("wqkvT", [n_layers, D, 3 * D], F32R, kind="ExternalInput")
    w1T = nc.dram_tensor("w1T", [n_layers, D, M], F32R, kind="ExternalInput")
    w2T = nc.dram_tensor("w2T", [n_layers, M, D], BF16, kind="ExternalInput")
    ln_g = nc.dram_tensor("ln_g", [2 * n_layers + 1, D], F32, kind="ExternalInput")
    ln_b = nc.dram_tensor("ln_b", [2 * n_layers + 1, D], F32, kind="ExternalInput")
    wunT = nc.dram_tensor("wunT", [D, V], F32R, kind="ExternalInput")
    bm1_in = nc.dram_tensor("bm1", [n_layers, M], F32, kind="ExternalInput")
    bm2_in = nc.dram_tensor("bm2", [n_layers, D], F32, kind="ExternalInput")
    bun = nc.dram_tensor("bun", [V, 1], F32, kind="ExternalInput")
    masks_in = nc.dram_tensor("masks", [4, 128, 512], BF16, kind="ExternalInput")
    sel2_in = nc.dram_tensor("sel2", [2, 128], F32R, kind="ExternalInput")
    ones_in = nc.dram_tensor("ones", [128, 128], F32R, kind="ExternalInput")
    uv_in = nc.dram_tensor("uv", [128, 3], F32, kind="ExternalInput")

    outT = nc.dram_tensor("outT", [V, TL], F32, kind="ExternalOutput")

    with tile.TileContext(nc) as tc:
        with (
            tc.tile_pool(name="per", bufs=1) as per,
            tc.tile_pool(name="big", bufs=1) as big,
            tc.tile_pool(name="kv", bufs=1) as kv,
            tc.tile_pool(name="stg", bufs=3) as stg,
            tc.tile_pool(name="strm", bufs=2) as strm,
            tc.tile_pool(name="sml", bufs=2) as sml,
            tc.tile_pool(name="wp", bufs=2) as wp,
            tc.tile_pool(name="esp", bufs=2) as esp,
            tc.tile_pool(name="ps_mm", bufs=2, space="PSUM") as ps_mm,
            tc.tile_pool(name="ps_pv", bufs=4, space="PSUM") as ps_pv,
            tc.tile_pool(name="ps_bc", bufs=2, space="PSUM") as ps_bc,
            tc.tile_pool(name="dram", bufs=2, space="DRAM") as dram,
        ):
            xT = per.tile([128, DC * TL], F32R, tag="xT")
            consts = per.tile([128, 128], F32R, tag="ones")
            nc.sync.dma_start(consts[:], ones_in.ap())
            sel2 = per.tile([2, 128], F32R, tag="sel2")
            nc.sync.dma_start(sel2[:], sel2_in.ap())
            uvw = per.tile([128, 3], F32, tag="uvw")
            nc.sync.dma_start(uvw[:], uv_in.ap())
            mask_sb = per.tile([128, 4 * 512], BF16, tag="masks")
            nc.sync.dma_start(
                mask_sb[:].rearrange("p (m t) -> p m t", m=4),
                masks_in.ap().rearrange("m p t -> p m t"),
            )
            ones_col = consts[:, 0:1]
            ones_row = consts[0:1, :]
            uvec = uvw[:, 0:1]
            eps_ap = uvw[0:1, 2:3]
            wvec = uvw[:, 1:2]

            def coef(h):
                return uvec if h == 0 else wvec

            def coefo(h):
                return wvec if h == 0 else uvec

            def ln_apply(src_fn, row):
                """LN over features (partition dim) via ones-matmul.
                src_fn(dc, tt) -> SBUF AP [128, 512] (f32r).
                Returns xn tile [128, 8192] f32r (big pool, tag 'xn')."""
                g_all = sml.tile([128, 8], F32, tag="g_all")
                b_all = sml.tile([128, 8], F32, tag="b_all")
                nc.sync.dma_start(
                    g_all[:],
                    ln_g.ap()[row : row + 1, :].rearrange("o (c p) -> (o p) c", p=128),
                )
                nc.sync.dma_start(
                    b_all[:],
                    ln_b.ap()[row : row + 1, :].rearrange("o (c p) -> (o p) c", p=128),
                )
                tA = sml.tile([1, TL], F32, tag="lnA", bufs=1)
                tB = sml.tile([1, TL], F32, tag="lnB", bufs=1)
                rstd_t = sml.tile([1, TL], F32R, tag="rstd", bufs=1)
                bb_t = sml.tile([1, TL], F32R, tag="bbt", bufs=1)
                for tt in range(2):
                    s1t = ps_bc.tile([128, 512], F32, tag="bc")
                    s2t = ps_bc.tile([128, 512], F32, tag="bc")
                    s1 = s1t[0:1, :]
                    s2 = s2t[0:1, :]
                    for dc in range(DC):
                        sl = src_fn(dc, tt)
                        nc.tensor.matmul(
                            s1, ones_col, sl, start=(dc == 0), stop=(dc == DC - 1)
                        )
                        sq = stg.tile([128, 512], F32R, tag="sq", bufs=2)
                        nc.scalar.square(sq[:], sl)
                        nc.tensor.matmul(
                            s2, ones_col, sq[:], start=(dc == 0), stop=(dc == DC - 1)
                        )
                    nc.scalar.copy(tA[0:1, tt * 512 : tt * 512 + 512], s1)
                    nc.scalar.copy(tB[0:1, tt * 512 : tt * 512 + 512], s2)
                # tA: sx -> mu ; tB: sxx -> msq -> var -> se
                nc.vector.tensor_scalar(tA[:], tA[:], 1.0 / D, None, op0=OP.mult)
                nc.vector.tensor_scalar(tB[:], tB[:], 1.0 / D, None, op0=OP.mult)
                with nc.allow_low_precision(reason="f32r musq"):
                    nc.vector.tensor_tensor(rstd_t[:], tA[:], tA[:], op=OP.mult)
                nc.vector.tensor_tensor(tB[:], tB[:], rstd_t[:], op=OP.subtract)
                nc.scalar.activation(tB[:], tB[:], AF.Sqrt, bias=eps_ap, scale=1.0)
                with nc.allow_low_precision(reason="f32r rstd"):
                    nc.vector.reciprocal(rstd_t[:], tB[:])
                nc.vector.scalar_tensor_tensor(
                    bb_t[:], tA[:], -1.0, rstd_t[:], op0=OP.mult, op1=OP.mult
                )
                xn = big.tile([128, DC * TL], F32R, tag="xn")
                for tt in range(2):
                    a_ps = ps_bc.tile([128, 512], F32, tag="bc")
                    b_ps = ps_bc.tile([128, 512], F32, tag="bc")
                    nc.tensor.matmul(
                        a_ps[:], ones_row, rstd_t[0:1, tt * 512 : tt * 512 + 512],
                        start=True, stop=True,
                    )
                    nc.tensor.matmul(
                        b_ps[:], ones_row, bb_t[0:1, tt * 512 : tt * 512 + 512],
                        start=True, stop=True,
                    )
                    for dc in range(DC):
                        sl = src_fn(dc, tt)
                        u1 = stg.tile([128, 512], F32, tag="u1", bufs=2)
                        nc.vector.tensor_tensor(u1[:], sl, a_ps[:], op=OP.mult)
                        nc.vector.tensor_tensor(u1[:], u1[:], b_ps[:], op=OP.add)
                        nc.vector.tensor_scalar(
                            xn[:, dc * TL + tt * 512 : dc * TL + tt * 512 + 512],
                            u1[:],
                            g_all[:, dc : dc + 1],
                            b_all[:, dc : dc + 1],
                            op0=OP.mult,
                            op1=OP.add,
                        )
                return xn

            def x_slice(dc, tt):
                return xT[:, dc * TL + tt * 512 : dc * TL + tt * 512 + 512]

            # ---------------- embed ----------------
            pos_sb = big.tile([128, DC * TL], F32R, tag="xn")
            nc.sync.dma_start(
                pos_sb[:].rearrange("p (c t) -> p c t", c=DC),
                posT.ap().rearrange("(c p) t -> p c t", p=128),
            )
            tok_sb = sml.tile([V, TL], F32R, tag="tok", bufs=1)
            nc.sync.dma_start(tok_sb[:], toksT.ap())
            for dc in range(DC):
                wt = wp.tile([128, 128], F32R, tag="wt")
                nc.sync.dma_start(wt[0:64, :], wtokT.ap()[:, dc * 128 : (dc + 1) * 128])
                for tt in range(2):
                    e_ps = ps_mm.tile([128, 512], F32, tag="mm")
                    nc.tensor.matmul(
                        e_ps[:], wt[0:64, :], tok_sb[:, tt * 512 : tt * 512 + 512],
                        start=True, stop=True,
                    )
                    sl = slice(dc * TL + tt * 512, dc * TL + tt * 512 + 512)
                    nc.vector.tensor_tensor(xT[:, sl], e_ps[:], pos_sb[:, sl], op=OP.add)

            # ---------------- layers ----------------
            for rep in range(repeat):
              if rep > 0:  # timing-only variant: keep values bounded
                for c in range(DC):
                    nc.vector.tensor_scalar(
                        xT[:, c * TL : (c + 1) * TL], xT[:, c * TL : (c + 1) * TL],
                        0.05, None, op0=OP.mult,
                    )
              for l in range(n_layers):
                xn = ln_apply(x_slice, 2 * l)

                ownQd = dram.tile([512, TL], BF16, tag="ownQd")
                ownYd = dram.tile([512, T], BF16, tag="ownYd")
                hTd = dram.tile([D, TL], F32R, tag="hTd")
                ag1_in = dram.tile([1536, TL], BF16, tag="ag1_in")
                ag1_out = dram.tile([2 * 1536, TL], BF16, tag="ag1_out")
                ag2_in = dram.tile([512, TL], BF16, tag="ag2_in")
                ag2_out = dram.tile([1024, TL], BF16, tag="ag2_out")

                KT = kv.tile([128, NP * T], BF16, tag="KT")
                Vsb = kv.tile([128, 16 * NLH * (HD + 1) + 128], BF16, tag="Vsb")
                vs4 = Vsb[:, 0 : 16 * NLH * (HD + 1)].rearrange("p (k h c) -> p k h c", k=16, c=HD + 1)
                nc.vector.memset(vs4[:, :, :, HD : HD + 1], 1.0)

                def rhs_xn(tt, _xn=xn):
                    return lambda dc: _xn[:, dc * TL + tt * 512 : dc * TL + tt * 512 + 512]

                def gemm_acc(out_ps, wdram, oc, rhs_of_dc, dt=F32R, _l=l):
                    wt = wp.tile([128, DC * 128], dt, tag="wt" if dt == F32R else "wt2")
                    nc.sync.dma_start(
                        wt[:].rearrange("p (c o) -> p c o", c=DC),
                        wdram.ap()[_l, :, oc * 128 : (oc + 1) * 128].rearrange(
                            "(c p) o -> p c o", p=128
                        ),
                    )
                    for dc in range(DC):
                        nc.tensor.matmul(
                            out_ps[:], wt[:, dc * 128 : (dc + 1) * 128], rhs_of_dc(dc),
                            start=(dc == 0), stop=(dc == DC - 1),
                        )

                # --- Q projection: all chunks -> DRAM (slotA ownQd, slotB ag1_in) ---
                for oc in range(8 if "qkv" not in skip else 0):
                    for tt in range(2):
                        q_ps = ps_mm.tile([128, 512], F32, tag="mm")
                        gemm_acc(q_ps, wqkvT, oc, rhs_xn(tt))
                        st = stg.tile([128, 512], BF16, tag="qstg")
                        nc.scalar.copy(st[:], q_ps[:])
                        if oc < 4:
                            dst = ownQd[128 * oc : 128 * oc + 128, tt * 512 : tt * 512 + 512]
                        else:
                            dst = ag1_in[128 * (oc - 4) : 128 * (oc - 4) + 128,
                                         tt * 512 : tt * 512 + 512]
                        nc.sync.dma_start(dst, st[:])
                # --- K projection: slotA dual-write to KT halves; slotB -> ag1_in ---
                for oc in range(8 if "qkv" not in skip else 0):
                    for tt in range(2):
                        k_ps = ps_mm.tile([128, 512], F32, tag="mm")
                        gemm_acc(k_ps, wqkvT, 8 + oc, rhs_xn(tt))
                        if oc < 4:
                            for h in range(2):
                                nc.vector.tensor_scalar(
                                    KT[:, 2048 * oc + 1024 * h + 512 * tt :
                                       2048 * oc + 1024 * h + 512 * tt + 512],
                                    k_ps[:], coef(h), None, op0=OP.mult,
                                )
                        else:
                            st = stg.tile([128, 512], BF16, tag="qstg")
                            nc.scalar.copy(st[:], k_ps[:])
                            nc.sync.dma_start(
                                ag1_in[512 + 128 * (oc - 4) : 512 + 128 * (oc - 4) + 128,
                                       tt * 512 : tt * 512 + 512],
                                st[:],
                            )
                # --- V projection (natural layout): lhsT = xn chunk, rhs = w ---
                for oh in range(2 if "qkv" not in skip else 0):
                    wtv = wp.tile([128, DC * 512], F32R, tag="wtv", bufs=1)
                    nc.sync.dma_start(
                        wtv[:].rearrange("p (c o) -> p c o", c=DC),
                        wqkvT.ap()[l, :, 2048 + oh * 512 : 2048 + oh * 512 + 512]
                        .rearrange("(c p) o -> p c o", p=128),
                    )
                    for tv in range(8):
                        v_ps = ps_mm.tile([128, 512], F32, tag="mm")
                        for dc in range(DC):
                            nc.tensor.matmul(
                                v_ps[:],
                                xn[:, dc * TL + tv * 128 : dc * TL + tv * 128 + 128],
                                wtv[:, dc * 512 : dc * 512 + 512],
                                start=(dc == 0),
                                stop=(dc == DC - 1),
                            )
                        vr = v_ps[:].rearrange("p (h c) -> p h c", h=NLH)
                        if oh == 0:
                            for h in range(2):
                                nc.vector.tensor_scalar(
                                    vs4[:, 8 * h + tv, :, 0:HD], vr, coef(h), None,
                                    op0=OP.mult,
                                )
                        else:
                            st = stg.tile([128, 512], BF16, tag="qstg")
                            nc.scalar.copy(st[:], v_ps[:])
                            vsec = ag1_in[1024:1536, :].rearrange(
                                "a (b c) -> (a b) c", b=2
                            )
                            nc.sync.dma_start(vsec[tv * 128 : tv * 128 + 128, :], st[:])

                if "coll" not in skip:
                    nc.gpsimd.collective_compute(
                        "AllGather", OP.bypass,
                        replica_groups=[[0, 1], [2, 3], [4, 5], [6, 7]],
                        ins=[ag1_in.opt()], outs=[ag1_out.opt()],
                    )

                # --- K/V assembly pass 2 (blob adds) ---
                for h in range(2 if "asm" not in skip else 0):
                    blob = 1536 * h
                    for p in range(NP):
                        kb = strm.tile([128, TL], BF16, tag="kb", bufs=1)
                        nc.sync.dma_start(
                            kb[:],
                            ag1_out[blob + 512 + 128 * p : blob + 512 + 128 * p + 128, :],
                        )
                        dsl = KT[:, 2048 * p + 1024 * h : 2048 * p + 1024 * h + 1024]
                        nc.vector.scalar_tensor_tensor(
                            dsl, kb[:], coefo(h), dsl, op0=OP.mult, op1=OP.add
                        )
                    vsec = ag1_out[blob + 1024 : blob + 1536, :].rearrange(
                        "a (b c) -> (a b) c", b=2
                    )
                    for kl in range(8):
                        vb = strm.tile([128, 512], BF16, tag="vb", bufs=1)
                        nc.sync.dma_start(vb[:], vsec[kl * 128 : kl * 128 + 128, :])
                        vbr = vb[:].rearrange("p (h c) -> p h c", h=NLH)
                        dst = vs4[:, 8 * h + kl, :, 0:HD]
                        nc.vector.scalar_tensor_tensor(
                            dst, vbr, coefo(h), dst, op0=OP.mult, op1=OP.add
                        )

                # --- attention ---
                # probe flags: attn_noexp (scores only), attn_nopv (no PV),
                # attn_notail (no rsp/rcp/rcb/yn tail)
                for q in range(NQT if "attn" not in skip else 0):
                    half = q // 2
                    qa = strm.tile([128, NP * 512], BF16, tag="qa", bufs=1)
                    nc.sync.dma_start(
                        qa[:].rearrange("p (g t) -> p g t", g=NP),
                        ownQd[:, 512 * (q % 2) : 512 * (q % 2) + 512].rearrange(
                            "(g p) t -> p g t", p=128
                        ),
                    )
                    qb = strm.tile([128, NP * 512], BF16, tag="qb", bufs=1)
                    nc.sync.dma_start(
                        qb[:].rearrange("p (g t) -> p g t", g=NP),
                        ag1_out[1536 * half : 1536 * half + 512,
                                512 * (q % 2) : 512 * (q % 2) + 512].rearrange(
                            "(g p) t -> p g t", p=128
                        ),
                    )
                    qt_t = strm.tile([128, NP * 512], BF16, tag="qt", bufs=1)
                    nc.vector.tensor_scalar(qt_t[:], qa[:], coef(half), None, op0=OP.mult)
                    nc.vector.scalar_tensor_tensor(
                        qt_t[:], qb[:], coefo(half), qt_t[:], op0=OP.mult, op1=OP.add
                    )
                    for p in range(NP):
                        pv_list = []
                        for _e in range(2):
                            pv0 = ps_pv.tile([128, 512], F32, tag="pv")
                            pv_list.append(pv0)
                        for k in range(4 * q + 4):
                            dlt = k - 4 * q
                            es_list = []
                            for e in range(2):
                                base = 64 * e
                                s_ps = ps_mm.tile([128, 512], F32, tag="mm")
                                nc.tensor.matmul(
                                    s_ps[:],
                                    KT[base : base + 64,
                                       2048 * p + 128 * k : 2048 * p + 128 * k + 128],
                                    qt_t[base : base + 64, 512 * p : 512 * p + 512],
                                    start=True, stop=True,
                                )
                                es = esp.tile([128, 512], BF16, tag="es", bufs=4)
                                if "attn_noexp" not in skip:
                                    nc.scalar.activation(es[:], s_ps[:], AF.Exp, scale=0.125)
                                    if dlt >= 0:
                                        nc.vector.tensor_tensor(
                                            es[:], es[:],
                                            mask_sb[:, 512 * dlt : 512 * dlt + 512],
                                            op=OP.mult,
                                        )
                                es_list.append(es)
                            if "attn_nopv" in skip or "attn_noexp" in skip:
                                continue
                            for e in range(2):
                                nc.tensor.matmul(
                                    pv_list[e][:],
                                    Vsb[:, 520 * k + 65 * (2 * p + e) :
                                        520 * k + 65 * (2 * p + e) + 128],
                                    es_list[e][:],
                                    start=(k == 0),
                                    stop=(k == 4 * q + 3),
                                )
                        if "attn_notail" in skip or "attn_nopv" in skip or "attn_noexp" in skip:
                            continue
                        rcp = sml.tile([1, 1024], F32R, tag="rcp", bufs=2)
                        with nc.allow_low_precision(reason="f32r softmax rcp"):
                            for e in range(2):
                                nc.vector.reciprocal(
                                    rcp[0:1, 512 * e : 512 * e + 512],
                                    pv_list[e][64:65, :],
                                )
                        rcb = stg.tile([128, 512], BF16, tag="rcb", bufs=3)
                        for e in range(2):
                            rcb_ps = ps_bc.tile([64, 512], F32, tag="bc")
                            nc.tensor.matmul(rcb_ps[:], ones_row[0:1, 0:64],
                                             rcp[0:1, 512 * e : 512 * e + 512],
                                             start=True, stop=True)
                            nc.scalar.copy(rcb[64 * e : 64 * e + 64, :], rcb_ps[:])
                        yn = stg.tile([128, 512], BF16, tag="yn", bufs=2)
                        for e in range(2):
                            nc.vector.tensor_tensor(
                                yn[64 * e : 64 * e + 64, :],
                                pv_list[e][0:64, :],
                                rcb[64 * e : 64 * e + 64, :],
                                op=OP.mult,
                            )
                        nc.sync.dma_start(
                            ownYd[128 * p : 128 * p + 128, 512 * q : 512 * q + 512], yn[:]
                        )

                # --- AG2: contribute my heads for partner tokens ---
                cst = big.tile([128, DC * TL], F32R, tag="xn")
                cstage = cst[:].bitcast(BF16)[:, : NP * TL]
                for p in range(NP):
                    oy0 = strm.tile([128, TL], BF16, tag="oy0", bufs=1)
                    nc.sync.dma_start(oy0[:], ownYd[128 * p : 128 * p + 128, 0:TL])
                    oy1 = strm.tile([128, TL], BF16, tag="oy1", bufs=1)
                    nc.sync.dma_start(oy1[:], ownYd[128 * p : 128 * p + 128, TL:T])
                    csl = cstage[:, p * TL : (p + 1) * TL]
                    nc.vector.tensor_scalar(csl, oy0[:], wvec, None, op0=OP.mult)
                    nc.vector.scalar_tensor_tensor(
                        csl, oy1[:], uvec, csl, op0=OP.mult, op1=OP.add
                    )
                nc.sync.dma_start(
                    ag2_in.rearrange("(g p) t -> p g t", p=128),
                    cstage.rearrange("p (g t) -> p g t", g=NP),
                )
                if "coll" not in skip:
                    nc.gpsimd.collective_compute(
                        "AllGather", OP.bypass,
                        replica_groups=[[0, 1], [2, 3], [4, 5], [6, 7]],
                        ins=[ag2_in.opt()], outs=[ag2_out.opt()],
                    )

                # --- h assembly: h = x + y -> hTd DRAM; x = 2x + y ---
                for c in range(DC):
                    low = c < 4
                    oyc = strm.tile([128, TL], BF16, tag="oyc", bufs=1)
                    nc.sync.dma_start(
                        oyc[:],
                        ownYd[128 * (c % 4) : 128 * (c % 4) + 128,
                              (0 if low else TL) : (TL if low else T)],
                    )
                    bbc = strm.tile([128, TL], BF16, tag="bbc", bufs=1)
                    nc.sync.dma_start(
                        bbc[:],
                        ag2_out[(0 if low else 512) + 128 * (c % 4) :
                                (0 if low else 512) + 128 * (c % 4) + 128, :],
                    )
                    nc.vector.tensor_scalar(
                        oyc[:], oyc[:], coef(0 if low else 1), None, op0=OP.mult
                    )
                    nc.vector.scalar_tensor_tensor(
                        bbc[:], bbc[:], coefo(0 if low else 1), oyc[:],
                        op0=OP.mult, op1=OP.add,
                    )
                    for tt in range(2):
                        hsl = stg.tile([128, 512], F32R, tag="u1", bufs=2)
                        xs = xT[:, c * TL + tt * 512 : c * TL + tt * 512 + 512]
                        ys = bbc[:, tt * 512 : tt * 512 + 512]
                        nc.vector.tensor_tensor(hsl[:], xs, ys, op=OP.add)
                        nc.sync.dma_start(
                            hTd[c * 128 : c * 128 + 128, tt * 512 : tt * 512 + 512], hsl[:]
                        )
                        nc.vector.scalar_tensor_tensor(
                            xs, xs, 2.0, ys, op0=OP.mult, op1=OP.add
                        )

                def h_slice(dc, tt, _h=hTd):
                    t_ = stg.tile([128, 512], F32R, tag="lnsrc", bufs=2)
                    nc.sync.dma_start(
                        t_[:], _h[dc * 128 : dc * 128 + 128, tt * 512 : tt * 512 + 512]
                    )
                    return t_[:]

                xn2 = ln_apply(h_slice, 2 * l + 1)

                # --- MLP ---
                def rhs_xn2(tt, _x=xn2):
                    return lambda dc: _x[:, dc * TL + tt * 512 : dc * TL + tt * 512 + 512]

                bm1_sb = sml.tile([128, 32], F32, tag="bm1")
                nc.sync.dma_start(
                    bm1_sb[:],
                    bm1_in.ap()[l : l + 1, :].rearrange("o (c p) -> (o p) c", p=128),
                )
                bm2_sb = sml.tile([128, 8], F32, tag="bm2")
                nc.sync.dma_start(
                    bm2_sb[:],
                    bm2_in.ap()[l : l + 1, :].rearrange("o (c p) -> (o p) c", p=128),
                )
                for tt in range(2 if "mlp" not in skip else 0):
                    h2acc = kv.tile([128, 8 * 512], F32, tag="KT")
                    for mb in range(4):
                        zblk = kv.tile([128, 8 * 512], BF16, tag="Vsb")
                        for mc in range(8):
                            z_ps = ps_mm.tile([128, 512], F32, tag="mm")
                            gemm_acc(z_ps, w1T, 8 * mb + mc, rhs_xn2(tt))
                            nc.scalar.activation(
                                zblk[:, 512 * mc : 512 * mc + 512], z_ps[:], AF.Relu,
                                bias=bm1_sb[:, 8 * mb + mc : 8 * mb + mc + 1], scale=1.0,
                            )
                        for oc in range(8):
                            d_ps = ps_mm.tile([128, 512], F32, tag="mm")
                            wt2 = wp.tile([128, 8 * 128], BF16, tag="wt2")
                            nc.sync.dma_start(
                                wt2[:].rearrange("p (c o) -> p c o", c=8),
                                w2T.ap()[l, 1024 * mb : 1024 * mb + 1024,
                                         oc * 128 : (oc + 1) * 128].rearrange(
                                    "(c p) o -> p c o", p=128
                                ),
                            )
                            for mc in range(8):
                                nc.tensor.matmul(
                                    d_ps[:], wt2[:, mc * 128 : mc * 128 + 128],
                                    zblk[:, 512 * mc : 512 * mc + 512],
                                    start=(mc == 0), stop=(mc == 7),
                                )
                            if mb == 0:
                                nc.scalar.activation(
                                    h2acc[:, 512 * oc : 512 * oc + 512], d_ps[:],
                                    AF.Identity,
                                    bias=bm2_sb[:, oc : oc + 1], scale=1.0,
                                )
                            else:
                                nc.vector.tensor_tensor(
                                    h2acc[:, 512 * oc : 512 * oc + 512],
                                    h2acc[:, 512 * oc : 512 * oc + 512],
                                    d_ps[:], op=OP.add,
                                )
                    for oc in range(8):
                        sl = slice(oc * TL + tt * 512, oc * TL + tt * 512 + 512)
                        nc.vector.tensor_tensor(
                            xT[:, sl], xT[:, sl], h2acc[:, 512 * oc : 512 * oc + 512],
                            op=OP.add,
                        )

            # ---------------- final LN + unembed ----------------
            xnf = ln_apply(x_slice, 2 * n_layers)
            bun_sb = sml.tile([V, 1], F32, tag="bun", bufs=1)
            nc.sync.dma_start(bun_sb[:], bun.ap())
            out_sb = sml.tile([V, TL], F32, tag="tok", bufs=1)
            for tt in range(2):
                o_ps = ps_mm.tile([128, 512], F32, tag="mm")
                for dc in range(DC):
                    wt = wp.tile([128, 128], F32R, tag="wt")
                    nc.sync.dma_start(wt[:, 0:V], wunT.ap()[dc * 128 : (dc + 1) * 128, :])
                    nc.tensor.matmul(
                        o_ps[0:V, :], wt[:, 0:V],
                        xnf[:, dc * TL + tt * 512 : dc * TL + tt * 512 + 512],
                        start=(dc == 0), stop=(dc == DC - 1),
                    )
                nc.scalar.activation(
                    out_sb[:, tt * 512 : tt * 512 + 512], o_ps[0:V, :], AF.Identity,
                    bias=bun_sb[:, 0:1], scale=1.0,
                )
            nc.sync.dma_start(outT.ap(), out_sb[:])

    nc.compile()
    return nc


# ---------------- host side ----------------


def prep_inputs(inputs, n_layers=L):
    f32 = np.float32
    toks = np.asarray(inputs["toks"], f32)
    W_tok = np.asarray(inputs["W_tok"], f32)
    W_pos = np.asarray(inputs["W_pos"], f32)
    Wqkv = np.asarray(inputs["Wqkv"], f32)
    W1 = np.asarray(inputs["W1"], f32)
    W2 = np.asarray(inputs["W2"], f32)
    Wun = np.asarray(inputs["Wun"], f32)
    bun = np.asarray(inputs["bun"], f32)
    g1, be1 = np.asarray(inputs["g1"], f32), np.asarray(inputs["be1"], f32)
    g2, be2 = np.asarray(inputs["g2"], f32), np.asarray(inputs["be2"], f32)
    gf, bf = np.asarray(inputs["gf"], f32), np.asarray(inputs["bf"], f32)
    bm1 = np.asarray(inputs["bm1"], f32)
    bm2 = np.asarray(inputs["bm2"], f32)

    masks = np.zeros((4, 128, 512), np.float32)
    i = np.arange(128)[:, None]
    j = np.arange(512)[None, :]
    for d in range(4):
        masks[d] = (j >= 128 * d + i).astype(f32)
    sel2 = np.zeros((2, 128), f32)
    sel2[0, 0:64] = 1.0
    sel2[1, 64:128] = 1.0
    ones = np.ones((128, 128), f32)
    ln_g = np.zeros((2 * n_layers + 1, D), f32)
    ln_b = np.zeros((2 * n_layers + 1, D), f32)
    for l in range(n_layers):
        ln_g[2 * l], ln_b[2 * l] = g1[l], be1[l]
        ln_g[2 * l + 1], ln_b[2 * l + 1] = g2[l], be2[l]
    ln_g[2 * n_layers], ln_b[2 * n_layers] = gf, bf

    w1T = np.ascontiguousarray(W1[:n_layers].transpose(0, 2, 1))
    w2T = np.ascontiguousarray(W2[:n_layers].transpose(0, 2, 1)).astype(ml_dtypes.bfloat16)
    wunT = np.ascontiguousarray(Wun.T)
    wtokT = np.ascontiguousarray(W_tok.T)
    masks_bf = masks.astype(ml_dtypes.bfloat16)

    in_maps = []
    for c in range(NC):
        b, jj = c // 2, c % 2
        ho = list(range(8 * jj, 8 * jj + 8)) + list(range(8 * (1 - jj), 8 * (1 - jj) + 8))
        idx_q = np.concatenate([np.arange(192 * h, 192 * h + 64) for h in ho])
        perm = np.concatenate([idx_q, idx_q + 64, idx_q + 128])
        wqkvT = np.ascontiguousarray(Wqkv[:n_layers][:, perm, :].transpose(0, 2, 1))
        u = 1.0 if jj == 0 else 0.0
        uv = np.zeros((128, 3), f32)
        uv[:, 0] = u
        uv[:, 1] = 1.0 - u
        uv[:, 2] = EPS
        in_maps.append(
            {
                "toksT": np.ascontiguousarray(toks[b, TL * jj : TL * jj + TL, :].T),
                "posT": np.ascontiguousarray(W_pos[:, TL * jj : TL * jj + TL]),
                "wtokT": wtokT,
                "wqkvT": wqkvT,
                "w1T": w1T,
                "w2T": w2T,
                "ln_g": ln_g,
                "ln_b": ln_b,
                "wunT": wunT,
                "bm1": bm1[:n_layers],
                "bm2": bm2[:n_layers],
                "bun": bun.reshape(V, 1),
                "masks": masks_bf,
                "sel2": sel2,
                "ones": ones,
                "uv": uv,
            }
        )
    return in_maps


def kernel(**inputs):
    if "prog" not in _CACHE:
        _CACHE["prog"] = build_program()
    nc = _CACHE["prog"]
    in_maps = prep_inputs(inputs)
    res = run_bass_kernel_spmd(nc, in_maps, list(range(NC)))
    out = np.zeros((B, T, V), np.float32)
    for c in range(NC):
        b, jj = c // 2, c % 2
        out[b, TL * jj : TL * jj + TL, :] = res.results[c]["outT"].T
    return out

